# revision 12
# baseline (speedup 1.0000x reference)
"""NeighborCorrelator Trainium2 kernel.

Math: xn = x/||x||_C, yn = y/||y||_C (per-pixel channel L2 norm, clamped at
1e-12); out[b, o=(i,j), h, w] = sum_c xn[b,c,h,w] * ynp[b,c,h+i,w+j] where
ynp is yn zero-padded by 3 on each spatial side. K=7 -> 49 offsets.
Shapes: x,y [4, 256, 256, 256] f32 -> out [4, 49, 256, 256] f32.

Strategy (8 NeuronCores, data-parallel over (batch, H-half)):
  - Each core gets x shard [256, 128, 256] and a zero-padded y halo slab
    [256, 134, 262] (H halo of 3 + W pad of 3, materialized on host).
  - On device (bf16): y stays resident in SBUF (loaded once, in 8-row
    chunks so compute starts early). For each 8x16 pixel patch, TensorE
    computes the cross-correlation band with four concurrent col-tiled
    matmuls (tile_position=(0,32j)): col tile j covers patch pixel rows
    {2j, 2j+1} and streams only its 8x22=176-column y window, so PSUM holds
    the band already trimmed to the useful 176 columns per 32-pixel quad.
    C=256 is accumulated as two K=128 matmuls per tile.
    The 49 useful offsets per pixel live at sheared positions
    t = (dh+i)*22 + (dw+j) - 44*(dh//2) of the 176-wide trimmed band.
  - Host: computes the per-pixel rsqrt channel norms of x and y in f32,
    gathers the sheared stencil out of the bf16 bands, multiplies by the
    norms; assembles [4, 49, 256, 256].
"""
import os
import sys

sys.path.insert(0, '/opt/trn_rl_repo')

import numpy as np
import ml_dtypes

import concourse.bass as bass
import concourse.bacc as bacc
import concourse.tile as tile
from concourse import mybir
from concourse.bass_utils import run_bass_kernel_spmd

B, C, H, W = 4, 256, 256, 256
K = 7
PAD = K // 2
EPS = 1e-12
NCORES = 8
HL = H // 2            # 128 rows per core
YH, YW = HL + 2 * PAD, W + 2 * PAD   # 134, 262

# patch geometry
PH, PW = 8, 16         # stationary patch (M = 128 pixels)
WW = PW + 2 * PAD      # y window row width 22
QH = PH + 2 * PAD - 6  # rows streamed per col tile quad: 8
NTRIM = QH * WW        # 176 trimmed band cols per quad
PTHG = HL // PH        # 16 patch rows per core
PTW = W // PW          # 16 patches per row
NPATCH = PTHG * PTW    # 256 per core
YCH = 32               # y rows per load chunk
NYCH = (YH + YCH - 1) // YCH         # 5 chunks (last has 6 rows)
XCH = 2                # patch rows per x load chunk
NXCH = PTHG // XCH     # 8 chunks

BF16 = mybir.dt.bfloat16
F32 = mybir.dt.float32

_CACHED_NC = None


def _build():
    nc = bacc.Bacc("TRN2", target_bir_lowering=False)
    x_d = nc.dram_tensor("x", [C, NPATCH, 128], BF16, kind="ExternalInput")
    y_d = nc.dram_tensor("y", [C, YH, YW], BF16, kind="ExternalInput")
    bands_d = nc.dram_tensor("bands", [PTHG, 128, PTW, NTRIM], BF16,
                             kind="ExternalOutput")

    def dma_y_chunk(y16, g):
        r0 = g * YCH
        nr = min(YCH, YH - r0)
        src = bass.AP(
            tensor=y_d, offset=r0 * YW,
            ap=[[YH * YW, 128], [128 * YH * YW, 2], [1, nr * YW]])
        # y goes on the scalar HWDGE ring: never queues behind an x-load's
        # buffer-reuse semaphore wait on the sync ring
        nc.scalar.dma_start(out=y16[:, :, r0:r0 + nr, :], in_=src)

    def dma_x_chunk(xt, c):
        # 2 patch rows per chunk: 8 KB contiguous runs per (c, ch)
        src = bass.AP(
            tensor=x_d, offset=c * XCH * PTW * 128,
            ap=[[NPATCH * 128, 128], [128 * NPATCH * 128, 2],
                [1, XCH * PTW * 128]])
        nc.sync.dma_start(out=xt, in_=src)

    with tile.TileContext(nc) as tc:
        with tc.tile_pool(name="ybuf", bufs=1) as ybufp, \
             tc.tile_pool(name="xchunk", bufs=3) as xchp, \
             tc.tile_pool(name="bandst", bufs=3) as bandp, \
             tc.tile_pool(name="ps", bufs=8, space="PSUM") as psp:

            y16 = ybufp.tile([128, 2, YH, YW], BF16, tag="y16")
            ypp = y16[:].ap[0][0]

            # loads all on sync HWDGE in consumption order; band stores go
            # out via the scalar HWDGE ring so they never block loads
            dma_y_chunk(y16, 0)
            xts = {}
            for c in range(2):
                xts[c] = xchp.tile([128, 2, XCH * PTW * 128], BF16, tag="xch",
                                   name=f"xch{c}")
                dma_x_chunk(xts[c], c)
            dma_y_chunk(y16, 1)

            for c in range(NXCH):
                if c + 2 < NXCH:
                    xts[c + 2] = xchp.tile([128, 2, XCH * PTW * 128], BF16,
                                           tag="xch", name=f"xch{c + 2}")
                    dma_x_chunk(xts[c + 2], c + 2)
                if c % 2 == 1 and c // 2 + 2 < NYCH:
                    dma_y_chunk(y16, c // 2 + 2)
                xt = xts.pop(c)

                for r in range(XCH):
                    ph = c * XCH + r
                    bst = bandp.tile([128, PTW, NTRIM], BF16, tag="bst")
                    for pw in range(PTW):
                        ps = psp.tile([128, NTRIM], F32, tag="band")
                        for ch in range(2):
                            for j in range(4):
                                lhsT = xt[:, ch, (r * PTW + pw) * 128 + 32 * j:
                                          (r * PTW + pw) * 128 + 32 * (j + 1)]
                                # y rows ph*8+2j .. +8, cols pw*16 .. +22
                                rhs = bass.AP(
                                    tensor=y16.tensor,
                                    offset=(y16.offset + ch * YH * YW
                                            + (ph * PH + 2 * j) * YW + pw * PW),
                                    ap=[[ypp, 128], [YW, QH], [1, WW]])
                                nc.tensor.matmul(ps[32 * j:32 * (j + 1), :],
                                                 lhsT, rhs,
                                                 start=(ch == 0), stop=(ch == 1),
                                                 tile_position=(0, 32 * j))
                        if pw % 2 == 0:
                            nc.vector.tensor_copy(out=bst[:, pw, :], in_=ps)
                        else:
                            nc.scalar.copy(out=bst[:, pw, :], in_=ps)
                    # one 5.6KB-per-partition store: [128, PTW*NTRIM] contig
                    bstpp = bst[:].ap[0][0]
                    src = bass.AP(tensor=bst.tensor, offset=bst.offset,
                                  ap=[[bstpp, 128], [1, PTW * NTRIM]])
                    dst = bass.AP(tensor=bands_d,
                                  offset=ph * 128 * PTW * NTRIM,
                                  ap=[[PTW * NTRIM, 128], [1, PTW * NTRIM]])
                    nc.scalar.dma_start(out=dst, in_=src)

    nc.finalize()
    return nc


def _host_gather(bands, rnx, rny):
    """bands [NPATCH,128,NTRIM] bf16 (patch-major), rnx [HL,W] f32,
    rny [YH,YW] f32 -> out core shard [49, HL, W] f32"""
    dh = np.arange(PH)[:, None, None, None]
    dw = np.arange(PW)[None, :, None, None]
    ii = np.arange(K)[None, None, :, None]
    jj = np.arange(K)[None, None, None, :]
    m_idx = np.broadcast_to(dh * PW + dw, (PH, PW, K, K)).reshape(-1)
    n_idx = ((dh + ii) * WW + (dw + jj) - 44 * (dh // 2)).reshape(-1)

    ext = bands[:, m_idx, n_idx].astype(np.float32)      # [NPATCH, PH*PW*49]
    ext = ext.reshape(PTHG, PTW, PH, PW, K, K)
    # -> [K, K, PTHG, PH, PTW, PW] -> [49, HL, W]
    ext = ext.transpose(4, 5, 0, 2, 1, 3).reshape(K * K, HL, W)

    rny_win = np.lib.stride_tricks.sliding_window_view(rny, (HL, W))  # [7,7,HL,W]
    ext *= rnx[None]
    ext *= rny_win.reshape(K * K, HL, W)
    return ext


def kernel(x: np.ndarray, y: np.ndarray) -> np.ndarray:
    global _CACHED_NC
    if _CACHED_NC is None:
        _CACHED_NC = _build()
    nc = _CACHED_NC

    x = np.ascontiguousarray(x, dtype=np.float32)
    y = np.ascontiguousarray(y, dtype=np.float32)

    # host-side f32 rsqrt channel norms (device only computes raw dots)
    ssx = np.einsum('bcp,bcp->bp', x.reshape(B, C, -1), x.reshape(B, C, -1))
    ssy = np.einsum('bcp,bcp->bp', y.reshape(B, C, -1), y.reshape(B, C, -1))
    rnx = 1.0 / np.maximum(np.sqrt(ssx.reshape(B, H, W)), EPS)
    rny = 1.0 / np.maximum(np.sqrt(ssy.reshape(B, H, W)), EPS)
    # zero-padded y grid: pad region has ss=0 -> rn=1/EPS, band there is 0
    rny_pad = np.full((B, H + 2 * PAD, W + 2 * PAD), 1.0 / EPS, dtype=np.float32)
    rny_pad[:, PAD:PAD + H, PAD:PAD + W] = rny

    x16h = x.astype(ml_dtypes.bfloat16)
    yp = np.zeros((B, C, H + 2 * PAD, YW), dtype=ml_dtypes.bfloat16)
    yp[:, :, PAD:PAD + H, PAD:PAD + W] = y.astype(ml_dtypes.bfloat16)

    in_maps = []
    for core in range(NCORES):
        b, half = divmod(core, 2)
        xs = x16h[b, :, half * HL:(half + 1) * HL, :]
        xs = xs.reshape(C, PTHG, PH, PTW, PW).transpose(0, 1, 3, 2, 4)
        xs = np.ascontiguousarray(xs.reshape(C, NPATCH, 128))
        ys = np.ascontiguousarray(yp[b, :, half * HL:half * HL + YH, :])
        in_maps.append({"x": xs, "y": ys})

    trace = bool(os.environ.get("BASS_TRACE"))
    if trace:
        try:
            from ntff_hook import install as _ihook
            _ihook()
        except Exception:
            try:
                _install_ntff_hook_inline()
            except Exception as e:
                print(f"(ntff hook unavailable: {e})", file=sys.stderr)

    res = run_bass_kernel_spmd(nc, in_maps, core_ids=list(range(NCORES)),
                               trace=trace)
    if res.exec_time_ns:
        print(f"HW exec time: {res.exec_time_ns} ns")

    out = np.empty((B, K * K, H, W), dtype=np.float32)
    for core in range(NCORES):
        b, half = divmod(core, 2)
        r = res.results[core]
        bands = r["bands"].view(ml_dtypes.bfloat16) if r["bands"].dtype != ml_dtypes.bfloat16 else r["bands"]
        # [PTHG, 128, PTW, NTRIM] -> patch-major [NPATCH, 128, NTRIM]
        bands = bands.reshape(PTHG, 128, PTW, NTRIM).transpose(0, 2, 1, 3)
        bands = bands.reshape(NPATCH, 128, NTRIM)
        out[b, :, half * HL:(half + 1) * HL, :] = _host_gather(
            bands, rnx[b, half * HL:(half + 1) * HL],
            rny_pad[b, half * HL:half * HL + YH])
    return out


def _install_ntff_hook_inline():
    import types
    import contextlib  # noqa
    mod = types.ModuleType("antenv.axon_hooks")
    _h = [None]
    mod.set_axon_ntff_profile_hook = lambda h: _h.__setitem__(0, h)
    mod.get_axon_ntff_profile_hook = lambda: _h[0]
    sys.modules["antenv.axon_hooks"] = mod
    import antenv
    antenv.axon_hooks = mod
    from trn_agent_boot.trn_boot import _ntff_profile_via_ctypes
    mod.set_axon_ntff_profile_hook(
        _ntff_profile_via_ctypes('/opt/axon/libaxon_pjrt.so'))


if __name__ == "__main__":
    rng = np.random.default_rng(0)
    xx = rng.standard_normal((B, C, H, W), dtype=np.float32)
    yy = rng.standard_normal((B, C, H, W), dtype=np.float32)
    o = kernel(x=xx, y=yy)
    print("out", o.shape, o.dtype)


# revision 14
# speedup vs baseline: 1.0606x; 1.0606x over previous
"""NeighborCorrelator Trainium2 kernel.

Math: xn = x/||x||_C, yn = y/||y||_C (per-pixel channel L2 norm, clamped at
1e-12); out[b, o=(i,j), h, w] = sum_c xn[b,c,h,w] * ynp[b,c,h+i,w+j] where
ynp is yn zero-padded by 3 on each spatial side. K=7 -> 49 offsets.
Shapes: x,y [4, 256, 256, 256] f32 -> out [4, 49, 256, 256] f32.

Strategy (8 NeuronCores, data-parallel over (batch, H-half)):
  - Each core gets x shard [256, 128, 256] and a zero-padded y halo slab
    [256, 134, 262] (H halo of 3 + W pad of 3, materialized on host).
  - On device (bf16): y stays resident in SBUF (loaded once, in 8-row
    chunks so compute starts early). For each 8x16 pixel patch, TensorE
    computes the cross-correlation band with four concurrent col-tiled
    matmuls (tile_position=(0,32j)): col tile j covers patch pixel rows
    {2j, 2j+1} and streams only its 8x22=176-column y window, so PSUM holds
    the band already trimmed to the useful 176 columns per 32-pixel quad.
    C=256 is accumulated as two K=128 matmuls per tile.
    The 49 useful offsets per pixel live at sheared positions
    t = (dh+i)*22 + (dw+j) - 44*(dh//2) of the 176-wide trimmed band.
  - Host: computes the per-pixel rsqrt channel norms of x and y in f32,
    gathers the sheared stencil out of the bf16 bands, multiplies by the
    norms; assembles [4, 49, 256, 256].
"""
import os
import sys

sys.path.insert(0, '/opt/trn_rl_repo')

import numpy as np
import ml_dtypes

import concourse.bass as bass
import concourse.bacc as bacc
import concourse.tile as tile
from concourse import mybir
from concourse.bass_utils import run_bass_kernel_spmd

B, C, H, W = 4, 256, 256, 256
K = 7
PAD = K // 2
EPS = 1e-12
NCORES = 8
HL = H // 2            # 128 rows per core
YH, YW = HL + 2 * PAD, W + 2 * PAD   # 134, 262

# patch geometry
PH, PW = 8, 16         # stationary patch (M = 128 pixels)
WW = PW + 2 * PAD      # y window row width 22
QH = PH + 2 * PAD - 6  # rows streamed per col tile quad: 8
NTRIM = QH * WW        # 176 trimmed band cols per quad
PTHG = HL // PH        # 16 patch rows per core
PTW = W // PW          # 16 patches per row
NPATCH = PTHG * PTW    # 256 per core
YCH = 32               # y rows per load chunk
NYCH = (YH + YCH - 1) // YCH         # 5 chunks (last has 6 rows)
XCH = 2                # patch rows per x load chunk
NXCH = PTHG // XCH     # 8 chunks

BF16 = mybir.dt.bfloat16
F32 = mybir.dt.float32

_CACHED_NC = None


def _build():
    nc = bacc.Bacc("TRN2", target_bir_lowering=False)
    x_d = nc.dram_tensor("x", [C, NPATCH, 128], BF16, kind="ExternalInput")
    y_d = nc.dram_tensor("y", [C, YH, YW], BF16, kind="ExternalInput")
    bands_d = nc.dram_tensor("bands", [PTHG, 128, PTW, NTRIM], BF16,
                             kind="ExternalOutput")

    def dma_y_chunk(y16, g):
        r0 = g * YCH
        nr = min(YCH, YH - r0)
        src = bass.AP(
            tensor=y_d, offset=r0 * YW,
            ap=[[YH * YW, 128], [128 * YH * YW, 2], [1, nr * YW]])
        nc.sync.dma_start(out=y16[:, :, r0:r0 + nr, :], in_=src)

    def dma_x_chunk(xt, c):
        # 2 patch rows per chunk: 8 KB contiguous runs per (c, ch)
        src = bass.AP(
            tensor=x_d, offset=c * XCH * PTW * 128,
            ap=[[NPATCH * 128, 128], [128 * NPATCH * 128, 2],
                [1, XCH * PTW * 128]])
        nc.sync.dma_start(out=xt, in_=src)

    with tile.TileContext(nc) as tc:
        with tc.tile_pool(name="ybuf", bufs=1) as ybufp, \
             tc.tile_pool(name="xchunk", bufs=3) as xchp, \
             tc.tile_pool(name="bandst", bufs=3) as bandp, \
             tc.tile_pool(name="ps", bufs=8, space="PSUM") as psp:

            y16 = ybufp.tile([128, 2, YH, YW], BF16, tag="y16")
            ypp = y16[:].ap[0][0]

            # loads all on sync HWDGE in consumption order; band stores go
            # out via the scalar HWDGE ring so they never block loads
            dma_y_chunk(y16, 0)
            xts = {}
            for c in range(2):
                xts[c] = xchp.tile([128, 2, XCH * PTW * 128], BF16, tag="xch",
                                   name=f"xch{c}")
                dma_x_chunk(xts[c], c)
            dma_y_chunk(y16, 1)

            for c in range(NXCH):
                # y first: it has no buffer-reuse wait, so it must not queue
                # behind the x-load's WAR semaphore on the sync ring
                if c % 2 == 0 and c // 2 + 2 < NYCH:
                    dma_y_chunk(y16, c // 2 + 2)
                if c + 2 < NXCH:
                    xts[c + 2] = xchp.tile([128, 2, XCH * PTW * 128], BF16,
                                           tag="xch", name=f"xch{c + 2}")
                    dma_x_chunk(xts[c + 2], c + 2)
                xt = xts.pop(c)

                for r in range(XCH):
                    ph = c * XCH + r
                    bst = bandp.tile([128, PTW, NTRIM], BF16, tag="bst")
                    for pw in range(PTW):
                        ps = psp.tile([128, NTRIM], F32, tag="band")
                        for ch in range(2):
                            for j in range(4):
                                lhsT = xt[:, ch, (r * PTW + pw) * 128 + 32 * j:
                                          (r * PTW + pw) * 128 + 32 * (j + 1)]
                                # y rows ph*8+2j .. +8, cols pw*16 .. +22
                                rhs = bass.AP(
                                    tensor=y16.tensor,
                                    offset=(y16.offset + ch * YH * YW
                                            + (ph * PH + 2 * j) * YW + pw * PW),
                                    ap=[[ypp, 128], [YW, QH], [1, WW]])
                                nc.tensor.matmul(ps[32 * j:32 * (j + 1), :],
                                                 lhsT, rhs,
                                                 start=(ch == 0), stop=(ch == 1),
                                                 tile_position=(0, 32 * j))
                        if pw % 2 == 0:
                            nc.vector.tensor_copy(out=bst[:, pw, :], in_=ps)
                        else:
                            nc.scalar.copy(out=bst[:, pw, :], in_=ps)
                    # one 5.6KB-per-partition store: [128, PTW*NTRIM] contig
                    bstpp = bst[:].ap[0][0]
                    src = bass.AP(tensor=bst.tensor, offset=bst.offset,
                                  ap=[[bstpp, 128], [1, PTW * NTRIM]])
                    dst = bass.AP(tensor=bands_d,
                                  offset=ph * 128 * PTW * NTRIM,
                                  ap=[[PTW * NTRIM, 128], [1, PTW * NTRIM]])
                    nc.scalar.dma_start(out=dst, in_=src)

    nc.finalize()
    return nc


def _host_gather(bands, rnx, rny):
    """bands [NPATCH,128,NTRIM] bf16 (patch-major), rnx [HL,W] f32,
    rny [YH,YW] f32 -> out core shard [49, HL, W] f32"""
    dh = np.arange(PH)[:, None, None, None]
    dw = np.arange(PW)[None, :, None, None]
    ii = np.arange(K)[None, None, :, None]
    jj = np.arange(K)[None, None, None, :]
    m_idx = np.broadcast_to(dh * PW + dw, (PH, PW, K, K)).reshape(-1)
    n_idx = ((dh + ii) * WW + (dw + jj) - 44 * (dh // 2)).reshape(-1)

    ext = bands[:, m_idx, n_idx].astype(np.float32)      # [NPATCH, PH*PW*49]
    ext = ext.reshape(PTHG, PTW, PH, PW, K, K)
    # -> [K, K, PTHG, PH, PTW, PW] -> [49, HL, W]
    ext = ext.transpose(4, 5, 0, 2, 1, 3).reshape(K * K, HL, W)

    rny_win = np.lib.stride_tricks.sliding_window_view(rny, (HL, W))  # [7,7,HL,W]
    ext *= rnx[None]
    ext *= rny_win.reshape(K * K, HL, W)
    return ext


def kernel(x: np.ndarray, y: np.ndarray) -> np.ndarray:
    global _CACHED_NC
    if _CACHED_NC is None:
        _CACHED_NC = _build()
    nc = _CACHED_NC

    x = np.ascontiguousarray(x, dtype=np.float32)
    y = np.ascontiguousarray(y, dtype=np.float32)

    # host-side f32 rsqrt channel norms (device only computes raw dots)
    ssx = np.einsum('bcp,bcp->bp', x.reshape(B, C, -1), x.reshape(B, C, -1))
    ssy = np.einsum('bcp,bcp->bp', y.reshape(B, C, -1), y.reshape(B, C, -1))
    rnx = 1.0 / np.maximum(np.sqrt(ssx.reshape(B, H, W)), EPS)
    rny = 1.0 / np.maximum(np.sqrt(ssy.reshape(B, H, W)), EPS)
    # zero-padded y grid: pad region has ss=0 -> rn=1/EPS, band there is 0
    rny_pad = np.full((B, H + 2 * PAD, W + 2 * PAD), 1.0 / EPS, dtype=np.float32)
    rny_pad[:, PAD:PAD + H, PAD:PAD + W] = rny

    x16h = x.astype(ml_dtypes.bfloat16)
    yp = np.zeros((B, C, H + 2 * PAD, YW), dtype=ml_dtypes.bfloat16)
    yp[:, :, PAD:PAD + H, PAD:PAD + W] = y.astype(ml_dtypes.bfloat16)

    in_maps = []
    for core in range(NCORES):
        b, half = divmod(core, 2)
        xs = x16h[b, :, half * HL:(half + 1) * HL, :]
        xs = xs.reshape(C, PTHG, PH, PTW, PW).transpose(0, 1, 3, 2, 4)
        xs = np.ascontiguousarray(xs.reshape(C, NPATCH, 128))
        ys = np.ascontiguousarray(yp[b, :, half * HL:half * HL + YH, :])
        in_maps.append({"x": xs, "y": ys})

    trace = bool(os.environ.get("BASS_TRACE"))
    if trace:
        try:
            from ntff_hook import install as _ihook
            _ihook()
        except Exception:
            try:
                _install_ntff_hook_inline()
            except Exception as e:
                print(f"(ntff hook unavailable: {e})", file=sys.stderr)

    res = run_bass_kernel_spmd(nc, in_maps, core_ids=list(range(NCORES)),
                               trace=trace)
    if res.exec_time_ns:
        print(f"HW exec time: {res.exec_time_ns} ns")

    out = np.empty((B, K * K, H, W), dtype=np.float32)
    for core in range(NCORES):
        b, half = divmod(core, 2)
        r = res.results[core]
        bands = r["bands"].view(ml_dtypes.bfloat16) if r["bands"].dtype != ml_dtypes.bfloat16 else r["bands"]
        # [PTHG, 128, PTW, NTRIM] -> patch-major [NPATCH, 128, NTRIM]
        bands = bands.reshape(PTHG, 128, PTW, NTRIM).transpose(0, 2, 1, 3)
        bands = bands.reshape(NPATCH, 128, NTRIM)
        out[b, :, half * HL:(half + 1) * HL, :] = _host_gather(
            bands, rnx[b, half * HL:(half + 1) * HL],
            rny_pad[b, half * HL:half * HL + YH])
    return out


def _install_ntff_hook_inline():
    import types
    import contextlib  # noqa
    mod = types.ModuleType("antenv.axon_hooks")
    _h = [None]
    mod.set_axon_ntff_profile_hook = lambda h: _h.__setitem__(0, h)
    mod.get_axon_ntff_profile_hook = lambda: _h[0]
    sys.modules["antenv.axon_hooks"] = mod
    import antenv
    antenv.axon_hooks = mod
    from trn_agent_boot.trn_boot import _ntff_profile_via_ctypes
    mod.set_axon_ntff_profile_hook(
        _ntff_profile_via_ctypes('/opt/axon/libaxon_pjrt.so'))


if __name__ == "__main__":
    rng = np.random.default_rng(0)
    xx = rng.standard_normal((B, C, H, W), dtype=np.float32)
    yy = rng.standard_normal((B, C, H, W), dtype=np.float32)
    o = kernel(x=xx, y=yy)
    print("out", o.shape, o.dtype)
